# revision 1
# baseline (speedup 1.0000x reference)
"""GATv2 (nn_GATv2_59184649339075) Bass kernel for TRN2, 8-core SPMD.

Self-contained: kernel(**inputs) takes the full unsharded inputs
(x[50000,64], W[64,64], b[64], a[64], edge_index[2,800000] int32) and
returns the full [50000,64] float32 output.

Strategy (edge-parallel with dst-tile ownership, no collectives):
  - Host: pad nodes to 51200 (400 tiles of 128). Sort edges by dst tile;
    each core owns 50 consecutive dst tiles (node-range sharded output).
    Within a tile, split edges into A (src < 25600) / B (src >= 25600) so
    dma_gather int16 indices cover the Wh table; sort each group by src
    for HBM locality; pad each group to fixed per-tile slot counts
    (compile-time max over all cores/tiles) so the SPMD program is static.
  - Device per core: Wh = x@W.T + b on-chip (bias folded in as an
    augmented contraction row), written to a DRAM table (256B rows) plus
    a per-core slice table for dst-side gathers.
    dma_gather (SWDGE, 4 queues round-robin, multi-packet) fetches Wh rows
    per edge by src (A/B calls) and by core-local dst.
    Edge-major score pipeline: s = Wh_dst+Wh_src (DVE), LeakyReLU (ACT
    Prelu alpha=0.2), *a + reduce (DVE), exp (ACT).
    One-hot [128e x 128n] f16 built via is_equal against an iota row;
    PE matmul onehot.T @ [exp*Wh_src | exp] (f16) accumulates numerator
    and denominator [128n, 65] in PSUM per dst tile.
    Output: sigmoid(numer * 1/denom) via ACT with per-partition scale.
"""
import sys

sys.path.insert(0, "/opt/trn_rl_repo")
from contextlib import ExitStack
from dataclasses import dataclass

import numpy as np

import concourse.bass as bass
import concourse.tile as tile
from concourse import bacc, mybir

F32 = mybir.dt.float32
F16 = mybir.dt.float16
I16 = mybir.dt.int16
I32 = mybir.dt.int32
AF = mybir.ActivationFunctionType

N_CORES = 8
P = 128
DIN = 64
DOUT = 64
NSLOPE = 0.2


@dataclass(frozen=True)
class GatCfg:
    n_pad: int          # padded node count, multiple of 128*8*gb_tiles
    sa: int             # per-tile A slots (multiple of 128)
    sb: int             # per-tile B slots
    gb_tiles: int       # tiles per gather super-batch

    @property
    def n_loc(self):
        return self.n_pad // N_CORES

    @property
    def tiles_core(self):
        return self.n_loc // P

    @property
    def half(self):
        return self.n_pad // 2

    @property
    def ca(self):
        return self.sa // P

    @property
    def cb(self):
        return self.sb // P

    @property
    def ct(self):
        return self.ca + self.cb

    @property
    def c_tot(self):
        return self.tiles_core * self.ct


def wrap16(idx):
    """Slot i of a gather call -> idx array position [i%16, i//16],
    replicated to the 128 partitions."""
    n = len(idx)
    assert n % 16 == 0
    a = idx.reshape(n // 16, 16).T.astype(np.int16)
    return np.tile(a, (8, 1))


def prepare(x, W, b, a, edge_index, gb_tiles=2):
    N = x.shape[0]
    E = edge_index.shape[1]
    blk = P * N_CORES * gb_tiles
    n_pad = ((N + blk - 1) // blk) * blk
    src = edge_index[0].astype(np.int64)
    dst = edge_index[1].astype(np.int64)
    half = n_pad // 2
    assert half <= 32768, "int16 gather indices require n_pad <= 65536"

    tile_id = dst >> 7
    grp = (src >= half).astype(np.int64)
    order = np.lexsort((src, grp, tile_id))
    src_s, dst_s, tile_s, grp_s = src[order], dst[order], tile_id[order], grp[order]

    n_tiles = n_pad // P
    tiles_core = n_tiles // N_CORES
    key = tile_s * 2 + grp_s
    counts = np.bincount(key, minlength=n_tiles * 2).reshape(n_tiles, 2)
    sa = max(int(np.ceil(counts[:, 0].max() / P) * P), P)
    sb = max(int(np.ceil(counts[:, 1].max() / P) * P), P)
    cfg = GatCfg(n_pad=n_pad, sa=sa, sb=sb, gb_tiles=gb_tiles)
    assert cfg.tiles_core % cfg.gb_tiles == 0

    rank_in_grp = np.arange(E) - np.repeat(
        np.concatenate([[0], np.cumsum(counts.reshape(-1))[:-1]]), counts.reshape(-1))
    core_of = tile_s // tiles_core
    t_in_core = tile_s % tiles_core
    batch = t_in_core // cfg.gb_tiles
    t_in_b = t_in_core % cfg.gb_tiles
    gb = cfg.gb_tiles
    batch_slots = gb * (sa + sb)
    slot = (batch * batch_slots
            + np.where(grp_s == 0,
                       t_in_b * sa + rank_in_grp,
                       gb * sa + t_in_b * sb + rank_in_grp))

    slots_core = tiles_core * (sa + sb)
    n_loc = cfg.n_loc
    n_loc_w = ((tiles_core + 7) // 8) * 8 * P

    xT = np.zeros((DIN + 1, n_pad), np.float32)
    xT[:DIN, :N] = x.T
    xT[DIN, :] = 1.0
    WT = np.concatenate([W.T.astype(np.float32),
                         b.reshape(1, DOUT).astype(np.float32)])
    a_row = a.reshape(1, DOUT).astype(np.float32)

    n_batches = tiles_core // gb
    fa_b = gb * sa // 16
    fb_b = gb * sb // 16
    fd_b = gb * (sa + sb) // 16

    in_maps = []
    for c in range(N_CORES):
        m = core_of == c
        s_src, s_dst, s_slot, s_grp = src_s[m], dst_s[m], slot[m], grp_s[m]
        srcA = np.zeros(slots_core, np.int16)
        srcB = np.zeros(slots_core, np.int16)
        dstL = np.zeros(slots_core, np.int16)
        dtl = np.full(slots_core, -1.0, np.float32)
        srcA[s_slot[s_grp == 0]] = s_src[s_grp == 0].astype(np.int16)
        srcB[s_slot[s_grp == 1]] = (s_src[s_grp == 1] - half).astype(np.int16)
        dstL[s_slot] = (s_dst - c * n_loc).astype(np.int16)
        dtl[s_slot] = (s_dst & (P - 1)).astype(np.float32)

        srcA_w = np.zeros((P, n_batches * fa_b), np.int16)
        srcB_w = np.zeros((P, n_batches * fb_b), np.int16)
        dstL_w = np.zeros((P, n_batches * fd_b), np.int16)
        for i in range(n_batches):
            lo = i * batch_slots
            srcA_w[:, i * fa_b:(i + 1) * fa_b] = wrap16(srcA[lo:lo + gb * sa])
            srcB_w[:, i * fb_b:(i + 1) * fb_b] = wrap16(
                srcB[lo + gb * sa:lo + batch_slots])
            dstL_w[:, i * fd_b:(i + 1) * fd_b] = wrap16(dstL[lo:lo + batch_slots])
        dtl_w = np.ascontiguousarray(dtl.reshape(cfg.c_tot, P).T)

        in_maps.append({
            "xT": xT,
            "xTs": np.ascontiguousarray(
                np.pad(xT[:, c * n_loc:(c + 1) * n_loc],
                       ((0, 0), (0, n_loc_w - n_loc)))),
            "WT": WT, "a": a_row,
            "srcA": srcA_w, "srcB": srcB_w, "dstL": dstL_w, "dtl": dtl_w,
        })
    return cfg, in_maps, {"N": N, "cfg": cfg}


def build(cfg: GatCfg, reps=1):
    nc = bacc.Bacc("TRN2", target_bir_lowering=False, debug=False,
                   num_devices=N_CORES, num_swdge_queues=4)
    n_pad, n_loc = cfg.n_pad, cfg.n_loc
    gb, sa, sb = cfg.gb_tiles, cfg.sa, cfg.sb
    ca, cb = cfg.ca, cfg.cb
    tiles_core = cfg.tiles_core
    n_batches = tiles_core // gb
    bc = gb * cfg.ct
    fa_b = gb * sa // 16
    fb_b = gb * sb // 16
    fd_b = gb * (sa + sb) // 16
    n_loc_w = ((tiles_core + 7) // 8) * 8 * P

    xT_d = nc.dram_tensor("xT", [DIN + 1, n_pad], F32, kind="ExternalInput").ap()
    xTs_d = nc.dram_tensor("xTs", [DIN + 1, n_loc_w], F32, kind="ExternalInput").ap()
    WT_d = nc.dram_tensor("WT", [DIN + 1, DOUT], F32, kind="ExternalInput").ap()
    a_d = nc.dram_tensor("a", [1, DOUT], F32, kind="ExternalInput").ap()
    srcA_d = nc.dram_tensor("srcA", [P, n_batches * fa_b], I16, kind="ExternalInput").ap()
    srcB_d = nc.dram_tensor("srcB", [P, n_batches * fb_b], I16, kind="ExternalInput").ap()
    dstL_d = nc.dram_tensor("dstL", [P, n_batches * fd_b], I16, kind="ExternalInput").ap()
    dtl_d = nc.dram_tensor("dtl", [P, cfg.c_tot], F32, kind="ExternalInput").ap()
    out_d = nc.dram_tensor("out", [n_loc, DOUT], F32, kind="ExternalOutput").ap()
    wh_d = nc.dram_tensor("wh", [n_pad, DOUT], F32).ap()
    whs_d = nc.dram_tensor("whs", [n_loc_w, DOUT], F32).ap()

    with tile.TileContext(nc) as tc:
        with ExitStack() as ctx:
            cpool = ctx.enter_context(tc.tile_pool(name="const", bufs=1))
            WT_sb = cpool.tile([DIN + 1, DOUT], F32)
            nc.sync.dma_start(WT_sb[:], WT_d[:, :])
            a_rep = cpool.tile([P, DOUT], F32)
            nc.sync.dma_start(a_rep[:], a_d.to_broadcast((P, DOUT)))
            iota_i = cpool.tile([P, P], I32)
            nc.gpsimd.iota(iota_i[:], pattern=[[1, P]], base=0, channel_multiplier=0)
            iota_f = cpool.tile([P, P], F32)
            nc.vector.tensor_copy(iota_f[:], iota_i[:])

            def wh_stage(ctx, src_ap, dst_ap, n_t):
                xp = ctx.enter_context(tc.tile_pool(name="xt", bufs=3))
                pp = ctx.enter_context(tc.tile_pool(name="whps", bufs=4, space="PSUM"))
                wp = ctx.enter_context(tc.tile_pool(name="whsb", bufs=3))
                GT = 8
                assert n_t % GT == 0
                for g in range(n_t // GT):
                    t0 = g * GT
                    xt = xp.tile([DIN + 1, GT * P], F32, tag="xt")
                    nc.sync.dma_start(xt[:], src_ap[:, t0 * P:(t0 + GT) * P])
                    ps = pp.tile([P, GT, DOUT], F32, tag="ps")
                    for j in range(GT):
                        nc.tensor.matmul(ps[:, j, :], lhsT=xt[:, j * P:(j + 1) * P],
                                         rhs=WT_sb[:], start=True, stop=True)
                    whb = wp.tile([P, GT, DOUT], F32, tag="whb")
                    nc.vector.tensor_copy(whb[:], ps[:])
                    nc.sync.dma_start(
                        dst_ap[t0 * P:(t0 + GT) * P, :]
                        .rearrange("(g p) f -> p g f", p=P), whb[:])

            with ExitStack() as c2:
                wh_stage(c2, xT_d, wh_d, n_pad // P)
                wh_stage(c2, xTs_d, whs_d, n_loc_w // P)

            gpool = ctx.enter_context(tc.tile_pool(name="gather", bufs=2))
            ipool = ctx.enter_context(tc.tile_pool(name="idx", bufs=2))
            spool = ctx.enter_context(tc.tile_pool(name="score", bufs=2))
            vpool = ctx.enter_context(tc.tile_pool(name="vals", bufs=2))
            opool = ctx.enter_context(tc.tile_pool(name="oh", bufs=2))
            apool = ctx.enter_context(tc.tile_pool(name="agg", bufs=4, space="PSUM"))
            npool = ctx.enter_context(tc.tile_pool(name="norm", bufs=4))

            for _ in range(reps):
                for bi in range(n_batches):
                    idxA = ipool.tile([P, fa_b], I16, tag="ia")
                    nc.sync.dma_start(idxA[:], srcA_d[:, bi * fa_b:(bi + 1) * fa_b])
                    idxB = ipool.tile([P, fb_b], I16, tag="ib")
                    nc.sync.dma_start(idxB[:], srcB_d[:, bi * fb_b:(bi + 1) * fb_b])
                    idxD = ipool.tile([P, fd_b], I16, tag="id")
                    nc.sync.dma_start(idxD[:], dstL_d[:, bi * fd_b:(bi + 1) * fd_b])
                    dtl = ipool.tile([P, bc], F32, tag="dtl")
                    nc.sync.dma_start(dtl[:], dtl_d[:, bi * bc:(bi + 1) * bc])

                    whsrc = gpool.tile([P, bc, DOUT], F32, tag="whsrc")
                    whdst = gpool.tile([P, bc, DOUT], F32, tag="whdst")
                    nA = gb * ca
                    q0 = (3 * bi) % 4
                    nc.gpsimd.dma_gather(
                        out_ap=whsrc[:, 0:nA, :], in_ap=wh_d[0:cfg.half, :],
                        idxs_ap=idxA[:], num_idxs=nA * P, num_idxs_reg=nA * P,
                        elem_size=DOUT, single_packet=False, queue_num=q0)
                    nc.gpsimd.dma_gather(
                        out_ap=whsrc[:, nA:bc, :], in_ap=wh_d[cfg.half:n_pad, :],
                        idxs_ap=idxB[:], num_idxs=gb * cb * P,
                        num_idxs_reg=gb * cb * P, elem_size=DOUT,
                        single_packet=False, queue_num=(q0 + 1) % 4)
                    nc.gpsimd.dma_gather(
                        out_ap=whdst[:, :, :], in_ap=whs_d[:, :],
                        idxs_ap=idxD[:], num_idxs=bc * P, num_idxs_reg=bc * P,
                        elem_size=DOUT, single_packet=False, queue_num=(q0 + 2) % 4)

                    s = spool.tile([P, bc, DOUT], F32, tag="s")
                    nc.vector.tensor_add(s[:], whsrc[:], whdst[:])
                    nc.scalar.activation(s[:], s[:], AF.Prelu, alpha=NSLOPE)
                    nc.vector.tensor_mul(s[:], s[:],
                                         a_rep[:].unsqueeze(1)
                                         .to_broadcast((P, bc, DOUT)))
                    e = spool.tile([P, bc], F32, tag="e")
                    nc.vector.tensor_reduce(e[:], s[:], axis=mybir.AxisListType.X,
                                            op=mybir.AluOpType.add)
                    ex = spool.tile([P, bc], F32, tag="ex")
                    nc.scalar.activation(ex[:], e[:], AF.Exp)

                    v = vpool.tile([P, bc, DOUT + 1], F16, tag="v")
                    nc.vector.tensor_mul(v[:, :, 0:DOUT], whsrc[:],
                                         ex[:].unsqueeze(2)
                                         .to_broadcast((P, bc, DOUT)))
                    nc.vector.tensor_copy(v[:, :, DOUT:DOUT + 1], ex[:].unsqueeze(2))

                    oh = opool.tile([P, bc, P], F16, tag="oh")
                    nc.vector.tensor_tensor(
                        oh[:],
                        dtl[:].unsqueeze(2).to_broadcast((P, bc, P)),
                        iota_f[:].unsqueeze(1).to_broadcast((P, bc, P)),
                        op=mybir.AluOpType.is_equal)

                    for tj in range(gb):
                        T = bi * gb + tj
                        ps = apool.tile([P, DOUT + 1], F32, tag="agg")
                        chunks = ([tj * ca + c for c in range(ca)]
                                  + [nA + tj * cb + c for c in range(cb)])
                        for k, c in enumerate(chunks):
                            nc.tensor.matmul(ps[:], lhsT=oh[:, c, :], rhs=v[:, c, :],
                                             start=(k == 0),
                                             stop=(k == len(chunks) - 1))
                        den = npool.tile([P, 1], F32, tag="den")
                        nc.vector.tensor_scalar_max(den[:], ps[:, DOUT:DOUT + 1], 1e-9)
                        rec = npool.tile([P, 1], F32, tag="rec")
                        nc.vector.reciprocal(rec[:], den[:])
                        ob = npool.tile([P, DOUT], F32, tag="ob")
                        nc.scalar.activation(ob[:], ps[:, 0:DOUT], AF.Sigmoid,
                                             scale=rec[:])
                        nc.sync.dma_start(out_d[T * P:(T + 1) * P, :], ob[:])

    nc.compile()
    return nc


_CACHE = {}


def kernel(x, W, b, a, edge_index):
    x = np.ascontiguousarray(np.asarray(x, dtype=np.float32))
    W = np.ascontiguousarray(np.asarray(W, dtype=np.float32))
    b = np.ascontiguousarray(np.asarray(b, dtype=np.float32))
    a = np.ascontiguousarray(np.asarray(a, dtype=np.float32))
    edge_index = np.asarray(edge_index)

    cfg, in_maps, meta = prepare(x, W, b, a, edge_index, gb_tiles=2)
    nc = _CACHE.get(cfg)
    if nc is None:
        nc = build(cfg)
        _CACHE[cfg] = nc

    from concourse.bass_utils import run_bass_kernel_spmd
    res = run_bass_kernel_spmd(nc, in_maps, core_ids=list(range(N_CORES)))
    parts = [res.results[c]["out"] for c in range(N_CORES)]
    return np.concatenate(parts, axis=0)[:meta["N"]].astype(np.float32)



# revision 9
# speedup vs baseline: 12.4463x; 12.4463x over previous
"""GATv2 (nn_GATv2_59184649339075) Bass kernel for TRN2, 8-core SPMD.

Self-contained: kernel(**inputs) takes the full unsharded inputs
(x[50000,64], W[64,64], b[64], a[64], edge_index[2,800000] int32) and
returns the full [50000,64] float32 output.

Design (v3, degree-bucketed dst-slot layout, no one-hots, no dst gather):
  Host: nodes are grouped into 400 dst-tiles of 128 by (per-node L/H edge
    counts); each core owns 50 tiles (tile rank r -> core r%8). Each dst
    node's edges occupy fixed columns of its tile: partition = dst node,
    column = edge slot, padded to per-position caps shared by all cores.
    The f16 node table holds 256B rows [a~*Wh | Wh] (a~ = |a| with the
    attention vector's sign folded into a feature reordering: F+ first).
    Table row r = p*400 + t; int16 gather indices address two overlapping
    windows (L: rows<32768, H: rows>=18432); per-tile partition placement
    puts high-src-degree nodes in the overlap so per-edge window choice
    can balance per-node L/H counts (pad factor ~1.08).
  Device per core: wh_stage computes the f16 table on PE (one [65,128]
    matmul per tile, bias folded) and writes it to DRAM; a second small
    pass computes the core's own 50 dst-tiles' scaled rows into SBUF.
    Edge phase per column-group: one dma_gather per window fetches
    [a~Wh|Wh] rows per edge; u = scaled_src + scaled_dst_tile (DVE f16
    2x, dst side broadcast by construction); Prelu on ACT; score
    e = sum(F+) - sum(F-) via pairwise f16 add-trees (2x); exp on ACT;
    v = raw_src * (masked exp) in-place (2x pair trick); per-tile
    aggregation and denominator via strided reduces; sigmoid(num/den)
    on ACT. Output layout [p, tile, f] f16, unpermuted on host.
"""
import sys

sys.path.insert(0, "/opt/trn_rl_repo")
from contextlib import ExitStack
from dataclasses import dataclass, field

import numpy as np

import concourse.bass as bass
import concourse.tile as tile
from concourse import bacc, mybir

F32 = mybir.dt.float32
F16 = mybir.dt.float16
I16 = mybir.dt.int16
AF = mybir.ActivationFunctionType

N_CORES = 8
P = 128
D = 64
NSLOPE = 0.2
N_TILES = 400
T_CORE = 50
NP_ = N_TILES * P          # 51200 padded nodes
LWIN = 32768               # L window rows [0, 32768)
HBASE = NP_ - 32768        # H window rows [18432, 51200)
GC = 120                   # max columns per edge group


@dataclass(frozen=True)
class Cfg:
    capL: tuple               # per-pos L caps (len 50)
    capH: tuple
    kpos: int                 # features with a >= 0 (F+ block size)
    groups: tuple             # tuple of (k0, k1) tile-pos ranges


def wrap16(idx):
    n = len(idx)
    assert n % 16 == 0
    a = idx.reshape(n // 16, 16).T.astype(np.int16)
    return np.tile(a, (8, 1))


def prepare(x, W, b, a, edge_index):
    N = x.shape[0]
    E = edge_index.shape[1]
    src = edge_index[0].astype(np.int64)
    dst = edge_index[1].astype(np.int64)

    deg = np.bincount(dst, minlength=NP_)
    sdeg = np.bincount(src, minlength=NP_)

    # --- phase 1: table partition per node (src side) -------------------
    order0 = np.argsort(-deg, kind="stable")
    tiles0 = order0.reshape(N_TILES, P)
    node_part = np.empty(NP_, np.int64)
    OVER = np.arange(47, 81)
    OTHER = np.array([p for p in range(P) if not (47 <= p <= 80)])
    so = np.argsort(-sdeg[tiles0], axis=1, kind="stable")
    for t in range(N_TILES):
        m = tiles0[t]
        o = so[t]
        node_part[m[o[:34]]] = OVER
        node_part[m[o[34:]]] = OTHER

    # --- per-edge window assignment (balance L/H per dst node) ----------
    src_p = node_part[src]
    canL = src_p <= 80
    canH = src_p >= 47
    free = canL & canH
    nLh = np.bincount(dst[canL & ~free], minlength=NP_)
    nHh = np.bincount(dst[canH & ~free], minlength=NP_)
    nF = np.bincount(dst[free], minlength=NP_)
    tot = nLh + nHh + nF
    nLb = np.maximum(nLh, np.minimum(nLh + nF, (tot + 1) // 2))
    nHb = tot - nLb
    # free edges of each dst: first (nLb - nLh) go L, rest H
    eorder = np.lexsort((~free, dst))   # per dst: free edges first
    e_sorted = np.arange(E)[eorder]
    d_sorted = dst[eorder]
    first = np.r_[True, d_sorted[1:] != d_sorted[:-1]]
    starts = np.flatnonzero(first)
    rank = np.arange(E) - np.repeat(starts, np.diff(np.r_[starts, E]))
    isfree_s = free[e_sorted]
    quotaL = (nLb - nLh)[d_sorted]
    toL_s = np.where(isfree_s, rank < quotaL, canL[e_sorted] )
    toL = np.empty(E, bool)
    toL[e_sorted] = toL_s
    assert (toL & ~canL).sum() == 0 and ((~toL) & ~canH).sum() == 0

    # --- phase 2: dst tiling + (pos, core) assignment -------------------
    key = np.lexsort((nHb, nLb, -tot))
    tiles = key.reshape(N_TILES, P)             # [tile, dstpos] -> node
    capL_t = nLb[tiles].max(1)
    capH_t = nHb[tiles].max(1)
    trank = np.argsort(-(capL_t + capH_t), kind="stable")
    # sorted position i -> pos i//8, core i%8
    tile_of = trank.reshape(T_CORE, N_CORES)     # [pos, core] -> tile id
    capL = capL_t[tile_of].max(1)
    capH = capH_t[tile_of].max(1)

    node_tile = np.empty(NP_, np.int64)
    node_dpos = np.empty(NP_, np.int64)
    for t in range(N_TILES):
        node_tile[tiles[t]] = t
        node_dpos[tiles[t]] = np.arange(P)

    # --- groups ---------------------------------------------------------
    groups = []
    k0 = 0
    while k0 < T_CORE:
        c = 0
        k1 = k0
        while k1 < T_CORE and (c == 0 or c + capL[k1] + capH[k1] <= GC):
            c += capL[k1] + capH[k1]
            k1 += 1
        groups.append((k0, k1))
        k0 = k1
    cfg_groups = tuple(groups)

    # --- feature reorder + sign fold ------------------------------------
    pos_f = np.flatnonzero(a >= 0)
    neg_f = np.flatnonzero(a < 0)
    fperm = np.concatenate([pos_f, neg_f])
    kpos = len(pos_f)
    atil = np.abs(a)[fperm]
    Wp = W[fperm]                  # [64 out-perm, 64 in]
    bp = b[fperm]
    WT_aug = np.zeros((D + 1, 2 * D), np.float16)
    WT_aug[:D, 0:D] = (Wp.T * atil).astype(np.float16)
    WT_aug[:D, D:2 * D] = Wp.T.astype(np.float16)
    WT_aug[D, 0:D] = (bp * atil).astype(np.float16)
    WT_aug[D, D:2 * D] = bp.astype(np.float16)

    cfg = Cfg(capL=tuple(int(v) for v in capL),
              capH=tuple(int(v) for v in capH),
              kpos=kpos, groups=cfg_groups)

    # --- per-core data ---------------------------------------------------
    # table t-index per (core, node): per core, per partition, nodes with
    # that partition get t = 0..399. Node's table column in xT = t*128+p.
    xpad = np.zeros((NP_, D), np.float32)
    xpad[:N] = x
    x16 = xpad.astype(np.float16)

    colsL = np.asarray(capL)
    colsH = np.asarray(capH)
    CC = int((colsL + colsH).sum())

    # edge sort: by (tile, window(toL first), dstpos, anything)
    in_maps = []
    # node table index t: shared across cores (no per-core constraint)
    t_of = np.empty(NP_, np.int64)
    for p in range(P):
        nodes_p = np.flatnonzero(node_part == p)
        assert len(nodes_p) == N_TILES
        t_of[nodes_p] = np.arange(N_TILES)
    row = node_part * N_TILES + t_of            # table row
    # xT column (t*128+p) -> node
    xcol = np.empty(NP_, np.int64)
    xcol[t_of * P + node_part] = np.arange(NP_)
    xT = np.ascontiguousarray(x16[xcol].T)      # [64, 51200] f16
    xT_aug = np.concatenate([xT, np.ones((1, NP_), np.float16)])

    # per-core edge slot tables
    e_tile = node_tile[dst]
    e_core = np.empty(E, np.int64)
    e_pos = np.empty(E, np.int64)
    # tile id -> (pos, core)
    tpos = np.empty(N_TILES, np.int64)
    tcore = np.empty(N_TILES, np.int64)
    for i in range(N_TILES):
        tcore[trank[i]] = i % N_CORES
        tpos[trank[i]] = i // N_CORES
    e_core = tcore[e_tile]
    e_pos = tpos[e_tile]
    e_dpos = node_dpos[dst]

    # column base offsets per (pos, window) in the group-local layout
    colbaseL = np.zeros(T_CORE, np.int64)
    colbaseH = np.zeros(T_CORE, np.int64)
    gstartL = {}
    gstartH = {}
    off = 0
    for (k0g, k1g) in cfg_groups:
        cL = int(colsL[k0g:k1g].sum())
        cH = int(colsH[k0g:k1g].sum())
        o = 0
        for k in range(k0g, k1g):
            colbaseL[k] = off + o
            o += colsL[k]
        for k in range(k0g, k1g):
            colbaseH[k] = off + o
            o += colsH[k]
        gstartL[k0g] = off
        gstartH[k0g] = off + cL
        off += cL + cH
    assert off == CC

    # per-core rank of edge within (dst node, window)
    for c in range(N_CORES):
        m = e_core == c
        ed = dst[m]
        es = src[m]
        eL = toL[m]
        ep = e_pos[m]
        edp = e_dpos[m]
        okey = np.lexsort((es, ~eL, ed))
        dk = ed[okey]; wk = eL[okey]
        bnd = np.r_[True, (dk[1:] != dk[:-1]) | (wk[1:] != wk[:-1])]
        st = np.flatnonzero(bnd)
        rk = np.arange(len(dk)) - np.repeat(st, np.diff(np.r_[st, len(dk)]))
        rank_e = np.empty(m.sum(), np.int64)
        rank_e[okey] = rk

        col = np.where(eL, colbaseL[ep] + rank_e, colbaseH[ep] + rank_e)
        slot = col * P + edp
        idx_full = np.zeros(CC * P, np.int64)          # default 0 (pad)
        r_e = row[es]
        idx_full[slot] = np.where(eL, r_e, r_e - HBASE)
        maskp = np.zeros((CC * P, 2), np.float16)
        maskp[slot] = 1.0

        idxL_parts = []
        idxH_parts = []
        for (k0g, k1g) in cfg_groups:
            cL = int(colsL[k0g:k1g].sum())
            cH = int(colsH[k0g:k1g].sum())
            s0 = gstartL[k0g] * P
            idxL_parts.append(wrap16(idx_full[s0:s0 + cL * P]))
            s1 = gstartH[k0g] * P
            idxH_parts.append(wrap16(idx_full[s1:s1 + cH * P]))
        idxL = np.concatenate(idxL_parts, axis=1) if idxL_parts else \
            np.zeros((P, 0), np.int16)
        idxH = np.concatenate(idxH_parts, axis=1) if idxH_parts else \
            np.zeros((P, 0), np.int16)

        # local dst-tile x (dst-arranged): columns = (pos k, dpos p)
        own_nodes = tiles[tile_of[:, c]].reshape(-1)   # [50*128]
        xloc = np.ascontiguousarray(x16[own_nodes].T)
        xloc_aug = np.concatenate([xloc, np.ones((1, T_CORE * P), np.float16)])

        in_maps.append({
            "xT": xT_aug, "xTloc": xloc_aug, "WT": WT_aug,
            "idxL": idxL, "idxH": idxH,
            "maskp": np.ascontiguousarray(
                maskp.reshape(CC, P, 2).transpose(1, 0, 2)).reshape(P, CC * 2),
        })

    meta = {"N": N, "fperm": fperm, "tiles": tiles, "tile_of": tile_of,
            "cfg": cfg}
    return cfg, in_maps, meta


def build(cfg: Cfg, reps=1):
    nc = bacc.Bacc("TRN2", target_bir_lowering=False, debug=False,
                   num_devices=N_CORES, num_swdge_queues=4)
    capL, capH = cfg.capL, cfg.capH
    groups = cfg.groups
    kpos = cfg.kpos
    CC = sum(capL) + sum(capH)

    xT_d = nc.dram_tensor("xT", [D + 1, NP_], F16, kind="ExternalInput").ap()
    xTl_d = nc.dram_tensor("xTloc", [D + 1, T_CORE * P], F16,
                           kind="ExternalInput").ap()
    WT_d = nc.dram_tensor("WT", [D + 1, 2 * D], F16, kind="ExternalInput").ap()
    idxL_d = nc.dram_tensor("idxL", [P, sum(capL) * 8], I16,
                            kind="ExternalInput").ap()
    idxH_d = nc.dram_tensor("idxH", [P, sum(capH) * 8], I16,
                            kind="ExternalInput").ap()
    maskp_d = nc.dram_tensor("maskp", [P, CC * 2], F16,
                             kind="ExternalInput").ap()
    out_d = nc.dram_tensor("out", [P, T_CORE * D], F16,
                           kind="ExternalOutput").ap()
    wh_t = nc.dram_tensor("wh", [P, N_TILES, 2 * D], F16)
    wh_d = wh_t.ap()
    wh_flat = wh_t.ap().rearrange("p t f -> (p t) f")

    def tree_sum(nc, pool, src_ap, cols, f0, f1, tag):
        """Pairwise f16 add-tree over feature range [f0, f1) of
        src_ap [P, cols, 64]; returns [P, cols] (scratch tile slice)."""
        n = f1 - f0
        cur = src_ap
        base = f0
        lvl = 0
        carry = None
        while n > 1:
            h = n // 2
            t = pool.tile([P, GC, max(h, 1)], F16, tag=f"{tag}l{lvl}")
            nc.vector.tensor_add(t[:, 0:cols, 0:h],
                                 cur[:, 0:cols, base:base + h],
                                 cur[:, 0:cols, base + h:base + 2 * h])
            if n % 2 == 1:
                if carry is None:
                    carry = (cur, base + 2 * h)
                else:
                    # add the old carry into the new tree's first column
                    nc.vector.tensor_add(t[:, 0:cols, 0:1],
                                         t[:, 0:cols, 0:1],
                                         carry[0][:, 0:cols,
                                                  carry[1]:carry[1] + 1])
                    carry = (cur, base + 2 * h)
            cur = t
            base = 0
            n = h
            lvl += 1
        out = pool.tile([P, GC], F16, tag=f"{tag}out")
        if carry is not None:
            nc.vector.tensor_add(out[:, 0:cols], cur[:, 0:cols, 0],
                                 carry[0][:, 0:cols, carry[1]])
        else:
            nc.vector.tensor_copy(out[:, 0:cols], cur[:, 0:cols, 0])
        return out

    with tile.TileContext(nc) as tc:
        with ExitStack() as ctx:
            cpool = ctx.enter_context(tc.tile_pool(name="const", bufs=1))
            WT_sb = cpool.tile([D + 1, 2 * D], F16)
            nc.sync.dma_start(WT_sb[:], WT_d[:, :])
            whloc = cpool.tile([P, T_CORE, D], F16)
            agg = cpool.tile([P, T_CORE, D], F32)
            den = cpool.tile([P, T_CORE], F32)
            rec = cpool.tile([P, T_CORE], F32)
            obuf = cpool.tile([P, T_CORE, D], F16)
            nc.vector.memset(agg[:], 0.0)
            nc.vector.memset(den[:], 0.0)

            # ---- wh_stage: full table + local scaled tiles -------------
            with ExitStack() as c2:
                xp = c2.enter_context(tc.tile_pool(name="xt", bufs=3))
                pp = c2.enter_context(tc.tile_pool(name="whps", bufs=3,
                                                   space="PSUM"))
                sp = c2.enter_context(tc.tile_pool(name="whsb", bufs=3))
                GT = 8
                for g in range(N_TILES // GT):
                    t0 = g * GT
                    xt = xp.tile([D + 1, GT * P], F16, tag="xt")
                    nc.sync.dma_start(xt[:], xT_d[:, t0 * P:(t0 + GT) * P])
                    ps = pp.tile([P, GT, 2 * D], F32, tag="ps")
                    for j in range(GT):
                        nc.tensor.matmul(ps[:, j, :],
                                         lhsT=xt[:, j * P:(j + 1) * P],
                                         rhs=WT_sb[:], start=True, stop=True)
                    st = sp.tile([P, GT, 2 * D], F16, tag="st")
                    if g % 2 == 0:
                        nc.vector.tensor_copy(st[:], ps[:])
                    else:
                        nc.scalar.activation(st[:], ps[:], AF.Identity)
                    nc.sync.dma_start(wh_d[:, t0:t0 + GT, :], st[:])
                # local pass: 50 tiles dst-arranged, keep scaled half
                for g in range(7):
                    t0 = g * GT
                    nt = min(GT, T_CORE - t0)
                    xt = xp.tile([D + 1, GT * P], F16, tag="xt")
                    nc.sync.dma_start(xt[:, 0:nt * P],
                                      xTl_d[:, t0 * P:(t0 + nt) * P])
                    ps = pp.tile([P, GT, 2 * D], F32, tag="ps")
                    for j in range(nt):
                        nc.tensor.matmul(ps[:, j, :],
                                         lhsT=xt[:, j * P:(j + 1) * P],
                                         rhs=WT_sb[:], start=True, stop=True)
                    if g % 2 == 0:
                        nc.vector.tensor_copy(whloc[:, t0:t0 + nt, :],
                                              ps[:, 0:nt, 0:D])
                    else:
                        nc.scalar.activation(whloc[:, t0:t0 + nt, :],
                                             ps[:, 0:nt, 0:D], AF.Identity)

            gp = ctx.enter_context(tc.tile_pool(name="gath", bufs=2))
            ip = ctx.enter_context(tc.tile_pool(name="idx", bufs=2))
            up = ctx.enter_context(tc.tile_pool(name="u", bufs=2))
            tp = ctx.enter_context(tc.tile_pool(name="tree", bufs=2))
            ssp = ctx.enter_context(tc.tile_pool(name="score", bufs=2))
            ap_ = ctx.enter_context(tc.tile_pool(name="aggs", bufs=2))

            for rep in range(reps):
                offL, offH, offC = 0, 0, 0
                for gi, (k0, k1) in enumerate(groups):
                    cL = sum(capL[k0:k1])
                    cH = sum(capH[k0:k1])
                    cols = cL + cH
                    iL = ip.tile([P, GC * 8], I16, tag="iL")
                    nc.sync.dma_start(iL[:, 0:cL * 8],
                                      idxL_d[:, offL * 8:(offL + cL) * 8])
                    iH = ip.tile([P, GC * 8], I16, tag="iH")
                    nc.sync.dma_start(iH[:, 0:cH * 8],
                                      idxH_d[:, offH * 8:(offH + cH) * 8])
                    mp = ip.tile([P, GC, 2], F16, tag="mp")
                    nc.sync.dma_start(
                        mp[:, 0:cols, :],
                        maskp_d[:, offC * 2:(offC + cols) * 2]
                        .rearrange("p (c r) -> p c r", r=2))

                    w = gp.tile([P, GC, 2 * D], F16, tag="w")
                    q0 = (3 * gi) % 4
                    nc.gpsimd.dma_gather(
                        out_ap=w[:, 0:cL, :], in_ap=wh_flat[0:LWIN, :],
                        idxs_ap=iL[:, 0:cL * 8], num_idxs=cL * P,
                        num_idxs_reg=cL * P, elem_size=2 * D,
                        single_packet=False, queue_num=q0)
                    nc.gpsimd.dma_gather(
                        out_ap=w[:, cL:cols, :], in_ap=wh_flat[HBASE:NP_, :],
                        idxs_ap=iH[:, 0:cH * 8], num_idxs=cH * P,
                        num_idxs_reg=cH * P, elem_size=2 * D,
                        single_packet=False, queue_num=(q0 + 1) % 4)

                    # u = scaled_src + scaled_dst (per tile piece)
                    u = up.tile([P, GC, D], F16, tag="u")
                    o = 0
                    for k in range(k0, k1):
                        if capL[k]:
                            nc.vector.tensor_add(
                                u[:, o:o + capL[k], :],
                                w[:, o:o + capL[k], 0:D],
                                whloc[:, k:k + 1, :]
                                .to_broadcast((P, capL[k], D)))
                        o += capL[k]
                    for k in range(k0, k1):
                        if capH[k]:
                            nc.vector.tensor_add(
                                u[:, o:o + capH[k], :],
                                w[:, o:o + capH[k], 0:D],
                                whloc[:, k:k + 1, :]
                                .to_broadcast((P, capH[k], D)))
                        o += capH[k]

                    nc.scalar.activation(u[:, 0:cols, :], u[:, 0:cols, :],
                                         AF.Prelu, alpha=NSLOPE)

                    eP = tree_sum(nc, tp, u, cols, 0, kpos, "tp")
                    eN = tree_sum(nc, tp, u, cols, kpos, D, "tn")
                    e = ssp.tile([P, GC], F16, tag="e")
                    nc.vector.tensor_sub(e[:, 0:cols], eP[:, 0:cols],
                                         eN[:, 0:cols])
                    ex = ssp.tile([P, GC], F16, tag="ex")
                    nc.scalar.activation(ex[:, 0:cols], e[:, 0:cols], AF.Exp)
                    ep = ssp.tile([P, GC, 2], F16, tag="ep")
                    nc.vector.tensor_mul(ep[:, 0:cols, :],
                                         ex[:, 0:cols].unsqueeze(2)
                                         .to_broadcast((P, cols, 2)),
                                         mp[:, 0:cols, :])

                    # v in-place over raw half
                    vr = w[:, 0:cols, D:2 * D].rearrange(
                        "p c (q r) -> p c q r", r=2)
                    nc.vector.tensor_mul(
                        vr, vr,
                        ep[:, 0:cols, :].unsqueeze(2)
                        .to_broadcast((P, cols, D // 2, 2)))

                    # per-tile aggregation + denominator
                    o = 0
                    written = set()
                    for cap in (capL, capH):
                        for k in range(k0, k1):
                            c = cap[k]
                            if c == 0:
                                continue
                            vv = w[:, o:o + c, D:2 * D].transpose((0, 2, 1))
                            if k not in written:
                                written.add(k)
                                nc.vector.tensor_reduce(
                                    agg[:, k, :], vv,
                                    axis=mybir.AxisListType.X,
                                    op=mybir.AluOpType.add)
                                nc.vector.tensor_reduce(
                                    den[:, k:k + 1], ep[:, o:o + c, :],
                                    axis=mybir.AxisListType.XY,
                                    op=mybir.AluOpType.add)
                            else:
                                t2 = ap_.tile([P, D], F32, tag="t2")
                                nc.vector.tensor_reduce(
                                    t2[:], vv, axis=mybir.AxisListType.X,
                                    op=mybir.AluOpType.add)
                                nc.vector.tensor_add(agg[:, k, :],
                                                     agg[:, k, :], t2[:])
                                d2 = ap_.tile([P, 1], F32, tag="d2")
                                nc.vector.tensor_reduce(
                                    d2[:], ep[:, o:o + c, :],
                                    axis=mybir.AxisListType.XY,
                                    op=mybir.AluOpType.add)
                                nc.vector.tensor_add(den[:, k:k + 1],
                                                     den[:, k:k + 1], d2[:])
                            o += c

                    offL += cL
                    offH += cH
                    offC += cols

                # tail: rec = 2 / max(den2, eps); den holds 2*den
                nc.vector.tensor_scalar_max(den[:], den[:], 1e-9)
                nc.vector.reciprocal(rec[:], den[:])
                nc.vector.tensor_scalar_mul(rec[:], rec[:], 2.0)
                for k in range(T_CORE):
                    nc.scalar.activation(obuf[:, k, :], agg[:, k, :],
                                         AF.Sigmoid, scale=rec[:, k:k + 1])
                nc.sync.dma_start(
                    out_d[:, :], obuf[:].rearrange("p t f -> p (t f)"))

    nc.compile()
    return nc


_CACHE = {}


def kernel(x, W, b, a, edge_index):
    x = np.ascontiguousarray(np.asarray(x, dtype=np.float32))
    W = np.ascontiguousarray(np.asarray(W, dtype=np.float32))
    b = np.ascontiguousarray(np.asarray(b, dtype=np.float32))
    a = np.ascontiguousarray(np.asarray(a, dtype=np.float32))
    edge_index = np.asarray(edge_index)

    cfg, in_maps, meta = prepare(x, W, b, a, edge_index)
    nc = _CACHE.get(cfg)
    if nc is None:
        nc = build(cfg)
        _CACHE[cfg] = nc

    from concourse.bass_utils import run_bass_kernel_spmd
    res = run_bass_kernel_spmd(nc, in_maps, core_ids=list(range(N_CORES)))

    N = meta["N"]
    fperm = meta["fperm"]
    tiles = meta["tiles"]
    tile_of = meta["tile_of"]
    inv_f = np.argsort(fperm)
    y = np.empty((NP_, D), np.float32)
    for c in range(N_CORES):
        o = np.asarray(res.results[c]["out"]).reshape(P, T_CORE, D)
        own = tiles[tile_of[:, c]]              # [50, 128]; o[p,k]=own[k,p]
        y[own.transpose(1, 0).reshape(-1)] = o.reshape(-1, D)
    return y[:N][:, inv_f].astype(np.float32)
